# revision 37
# baseline (speedup 1.0000x reference)
"""Trainium2 Bass kernel for CausalFlowModel (RNN scan + 2 MLPs + combinator).

Sharding: data-parallel over batch across 8 NeuronCores (64 rows/core).
All weights replicated, pre-transposed+packed on host into lhsT tile banks.
Everything on-device runs in bf16 with fp32 PSUM accumulation; biases are
folded into the matmuls via an appended ones-row (they are all zero for this
problem, but handled correctly anyway).

Layout convention: all activations live TRANSPOSED in SBUF as
[feature-partition, batch-column] so the 511-step recurrence needs no
per-step transposes:  hT_{t+1}[m-block] = tanh( sum_k WhT[k,m].T @ hT_t[k]
                                               + WuT_aug[m].T @ uT_aug_t )

RNN step schedule (the performance-critical part): each step's pre-activation
accumulates into TWO PSUM banks (bank A = h-regions 0..1, bank B = 2..3) so
the two tanh halves on ScalarE can each overlap TensorE work on the OTHER
bank.  The measured steady-state period (1072ns) sits ~30ns above the
structural floor 52(sem) + 115(4 gated issues) + 194(drain) + 90(sem) +
265(ScalarE ACT issue spacing) + 355(ACT dur): both tanhs serialize on
ScalarE and every bank needs both k-halves, so the floor is invariant to
slot order / bank split.  Slot order per step:
    u0(A) u1(A) u2(B) u3(B)                # independent of h, fills latency
    (m01,k01 -> A) (m23,k01 -> B)          # consume prev blocks 0,1 only
    (m01,k23 -> A)  [tanh A]  (m23,k23 -> B)  [tanh B]
Scheduling hygiene matters as much as the period: x_dnn work is order-pinned
to its drain step (Tile otherwise hoists it into the early RNN), the big
MLP-weight DMAs are sem-pinned to mid-RNN steps (their transfers otherwise
collide with the early RNN's SBUF traffic and trip the HAM re-throttle), and
dummy-matmul fills keep PE duty high across the warmup->RNN and RNN->tail
transitions.
"""

import numpy as np
import ml_dtypes

B, T = 512, 512
SD, CD, H = 256, 64, 512
D1, D2 = 1024, 1024
NCORES = 8
BL = B // NCORES          # 64 batch rows per core
CHUNK = 64                # u steps per DMA chunk
WARMUP_MM = 130           # dense dummy matmuls before the RNN; HAM flip to
                          # 2.4GHz measured ~4.5us (~85 cold MMs) after the
                          # first MM, worst-case phase needs ~6.8us (~128)
XDNN_T0 = 160             # RNN step where interleaved x_dnn work begins
XDNN_EVERY = 4            # drain one x_dnn item per this many steps: keeps
                          # the extra ScalarE/PE load too thin to disturb
                          # the HAM activity window
NSTEPS = T - 1            # 511 scan steps

_BF = ml_dtypes.bfloat16

_CACHE = {}


def _bf16(a):
    return np.ascontiguousarray(np.asarray(a, np.float32)).astype(_BF)


def _pack_kxm(W, n_m, n_k, k_off=0):
    """lhsT tile bank [128, n_k*n_m*128]; block j=k*n_m+m is
    W[m*128:(m+1)*128, k_off+k*128 : k_off+(k+1)*128].T"""
    cols = []
    for k in range(n_k):
        for m in range(n_m):
            cols.append(W[m * 128:(m + 1) * 128,
                          k_off + k * 128: k_off + (k + 1) * 128].T)
    return np.concatenate(cols, axis=1)


def _pack_head_bias(W, bvec, n_m, width):
    """[width+1, n_m*128]; block m = [W[m*128:(m+1)*128, :width].T ; b[mblock]]"""
    cols = []
    for m in range(n_m):
        blk = np.concatenate(
            [W[m * 128:(m + 1) * 128, :width].T,
             bvec[m * 128:(m + 1) * 128][None, :]], axis=0)
        cols.append(blk)
    return np.concatenate(cols, axis=1)


def _weight_arrays(inp):
    i2h_W, i2h_b = inp["i2h_W"], inp["i2h_b"]
    w = {
        "whT": _pack_kxm(i2h_W, 4, 4, k_off=CD),
        "wuT": _pack_head_bias(i2h_W, i2h_b, 4, CD),          # [65, 512]
        "x1T": _pack_kxm(inp["x1_W"], 8, 2, k_off=1),
        "x1tb": _pack_head_bias(inp["x1_W"], inp["x1_b"], 8, 1),  # [2, 1024]
        "x2T": _pack_kxm(inp["x2_W"], 8, 8),
        "x2b": np.asarray(inp["x2_b"], np.float32)[None, :],
        "x3T": _pack_kxm(inp["x3_W"], 2, 8),
        "x3b": np.asarray(inp["x3_b"], np.float32)[None, :],
        "u1T": _pack_kxm(inp["u1_W"], 8, 2, k_off=1),
        "u1tb": _pack_head_bias(inp["u1_W"], inp["u1_b"], 8, 1),
        "u2T": _pack_kxm(inp["u2_W"], 8, 8),
        "u2b": np.asarray(inp["u2_b"], np.float32)[None, :],
        "u3T": _pack_kxm(inp["u3_W"], 2, 8),
        "u3b": np.asarray(inp["u3_b"], np.float32)[None, :],
        "h2oT": _pack_kxm(inp["h2o_W"], 2, 4, k_off=CD),
        "h2o_uT": _pack_head_bias(inp["h2o_W"], inp["h2o_b"], 2, CD),  # [65, 256]
        "combT": _pack_kxm(inp["comb_W"], 2, 4),
        "combb": np.asarray(inp["comb_b"], np.float32)[None, :],
    }
    return {k: _bf16(v) for k, v in w.items()}


def _per_core_arrays(inp, c):
    t = np.asarray(inp["t"], np.float32)
    x = np.asarray(inp["x"], np.float32)
    u = np.asarray(inp["u"], np.float32)
    b0 = c * BL
    us = u[:, b0:b0 + BL, :].transpose(2, 0, 1).reshape(CD, T * BL)
    u_aug = np.concatenate([us, np.ones((1, T * BL), np.float32)], axis=0)
    xT = x[b0:b0 + BL].T                              # [256, BL]
    xt = np.concatenate([xT[:128], xT[128:]], axis=1)  # [128, 2*BL]
    tb = np.stack([t[b0:b0 + BL, 0], np.ones(BL, np.float32)], axis=0)  # [2, BL]
    return {"u_aug": _bf16(u_aug), "xt": _bf16(xt), "tb": _bf16(tb)}


def _build_program(debug=False):
    import concourse.bass as bass
    import concourse.mybir as mybir
    from concourse import bacc
    from concourse.tile import TileContext

    bf = mybir.dt.bfloat16
    f32 = mybir.dt.float32
    TANH = mybir.ActivationFunctionType.Tanh

    nc = bacc.Bacc("TRN2", target_bir_lowering=False, debug=False)

    d_in = {}
    def din(name, shape, dt=bf):
        d_in[name] = nc.dram_tensor(name, list(shape), dt, kind="ExternalInput")
        return d_in[name]

    u_aug_d = din("u_aug", (CD + 1, T * BL))
    xt_d = din("xt", (128, 2 * BL))
    tb_d = din("tb", (2, BL))
    wh_d = din("whT", (128, 16 * 128))
    wu_d = din("wuT", (CD + 1, 4 * 128))
    x1_d = din("x1T", (128, 16 * 128))
    x1tb_d = din("x1tb", (2, 8 * 128))
    x2_d = din("x2T", (128, 64 * 128))
    x2b_d = din("x2b", (1, 8 * 128))
    x3_d = din("x3T", (128, 16 * 128))
    x3b_d = din("x3b", (1, 2 * 128))
    u1_d = din("u1T", (128, 16 * 128))
    u1tb_d = din("u1tb", (2, 8 * 128))
    u2_d = din("u2T", (128, 64 * 128))
    u2b_d = din("u2b", (1, 8 * 128))
    u3_d = din("u3T", (128, 16 * 128))
    u3b_d = din("u3b", (1, 2 * 128))
    h2o_d = din("h2oT", (128, 8 * 128))
    h2ou_d = din("h2o_uT", (CD + 1, 2 * 128))
    comb_d = din("combT", (128, 8 * 128))
    combb_d = din("combb", (1, 2 * 128))
    out_d = nc.dram_tensor("out", [2 * 128, BL], f32, kind="ExternalOutput")
    dbg = {}
    if debug:
        for name in ("dbg_h0", "dbg_h1", "dbg_hlast"):
            dbg[name] = nc.dram_tensor(name, [128, 4 * BL], f32,
                                       kind="ExternalOutput")
        for name in ("dbg_r", "dbg_s", "dbg_c"):
            dbg[name] = nc.dram_tensor(name, [128, 2 * BL], f32,
                                       kind="ExternalOutput")

    with TileContext(nc) as tc:
        with (
            tc.tile_pool(name="consts", bufs=1) as consts,
            tc.tile_pool(name="upool", bufs=2) as upool,
            tc.tile_pool(name="hpool", bufs=6) as hpool,
            tc.tile_pool(name="work", bufs=1) as work,
        ):
            mm = nc.tensor.matmul
            # --- PE p-state warmup: dense dummy stream overlapping the ---
            # --- initial DMA wait; pushes the PE clock to 2.4GHz early ---
            warm_ctx = tc.tile_pool(name="warmps", bufs=1, space="PSUM")
            warmps = warm_ctx.__enter__()
            dummy = work.tile([128, 128], bf, name="dummy")
            nc.vector.memset(dummy[:, :], 0.0)
            wps = warmps.tile([128, 64], f32, name="wps")
            for _ in range(WARMUP_MM):
                mm(wps[:, :], dummy[:, :], dummy[:, 0:64],
                   start=True, stop=True, skip_group_check=True)

            def cload(dram, shape, dt=bf, name=None):
                tile = consts.tile(list(shape), dt, name=name)
                nc.sync.dma_start(out=tile[:, :], in_=dram[:, :])
                return tile

            # --- DMAs the RNN needs first, ordered so the scan starts ASAP:
            # a small head of u-chunk 0, then wu, then wh k-slices, then the
            # rest of chunk 0 ---
            u_tile = upool.tile([CD + 1, CHUNK * BL], bf, name="ut")
            head = 8 * BL
            nc.sync.dma_start(out=u_tile[:, 0:head], in_=u_aug_d[:, 0:head])
            wu_sb = cload(wu_d, (CD + 1, 4 * 128), name="wu_sb")
            wh_sb = consts.tile([128, 16 * 128], bf, name="wh_sb")
            for kk in range(4):
                nc.sync.dma_start(out=wh_sb[:, kk * 512:(kk + 1) * 512],
                                  in_=wh_d[:, kk * 512:(kk + 1) * 512])
            nc.sync.dma_start(out=u_tile[:, head:CHUNK * BL],
                              in_=u_aug_d[:, head:CHUNK * BL])
            # --- remaining consts (stream in during the RNN) ---
            tb_sb = cload(tb_d, (2, BL), name="tb_sb")
            ones_sb = consts.tile([1, BL], bf, name="ones_sb")
            nc.sync.dma_start(out=ones_sb[:, :], in_=tb_d[1:2, :])
            xt_sb = cload(xt_d, (128, 2 * BL), name="xt_sb")
            h2o_sb = cload(h2o_d, (128, 8 * 128), name="h2o_sb")
            h2ou_sb = cload(h2ou_d, (CD + 1, 2 * 128), name="h2ou_sb")

            # The MLP weights (~7.5MB, first needed at step XDNN_T0) are
            # DMAed mid-RNN, sem-pinned to step milestones: streaming them
            # at kernel start collides with the early RNN's SBUF traffic
            # (observed ~2us TE stalls -> HAM re-throttle).
            def dload(dram, shape, name):
                tile = consts.tile(list(shape), bf, name=name)

                def go(after):
                    inst = nc.sync.dma_start(out=tile[:, :], in_=dram[:, :])
                    if after is not None:
                        add_dep_helper(inst.ins, after.ins, sync=True,
                                       reason="delay big const DMA")
                return tile, go

            x1_sb, x1_go = dload(x1_d, (128, 16 * 128), name="x1_sb")
            x1tb_sb, x1tb_go = dload(x1tb_d, (2, 8 * 128), name="x1tb_sb")
            x2_sb, x2_go = dload(x2_d, (128, 64 * 128), name="x2_sb")
            x2b_sb, x2b_go = dload(x2b_d, (1, 8 * 128), name="x2b_sb")
            x3_sb, x3_go = dload(x3_d, (128, 16 * 128), name="x3_sb")
            x3b_sb, x3b_go = dload(x3b_d, (1, 2 * 128), name="x3b_sb")
            u1_sb, u1_go = dload(u1_d, (128, 16 * 128), name="u1_sb")
            u1tb_sb, u1tb_go = dload(u1tb_d, (2, 8 * 128), name="u1tb_sb")
            u2_sb, u2_go = dload(u2_d, (128, 64 * 128), name="u2_sb")
            u2b_sb, u2b_go = dload(u2b_d, (1, 8 * 128), name="u2b_sb")
            u3_sb, u3_go = dload(u3_d, (128, 16 * 128), name="u3_sb")
            u3b_sb, u3b_go = dload(u3b_d, (1, 2 * 128), name="u3b_sb")
            comb_sb, comb_go = dload(comb_d, (128, 8 * 128), name="comb_sb")
            combb_sb, combb_go = dload(combb_d, (1, 2 * 128), name="combb_sb")
            delayed_loads = {
                60: [x1_go, x1tb_go], 100: [x2_go], 150: [x2b_go, x3_go],
                190: [x3b_go, u1_go, u1tb_go], 230: [u2_go],
                280: [u2b_go, u3_go], 320: [u3b_go, comb_go, combb_go],
            }

            warm_ctx.__exit__(None, None, None)
            mlpps_ctx = tc.tile_pool(name="mlpps", bufs=2, space="PSUM")
            mlpps = mlpps_ctx.__enter__()


            def mlptile():
                return mlpps.tile([128, 4 * BL], f32, name="mlp")

            rnnps_ctx = tc.tile_pool(name="rnnps", bufs=3, space="PSUM")
            rnnps = rnnps_ctx.__enter__()

            # ---------------- RNN scan: 511 steps ----------------
            # The u-part matmuls for step t+2 are emitted at the END of
            # iteration t (explicit 2-deep software pipeline): they are the
            # only h-independent PE work, and placing them right after each
            # step's tail keeps the PE busy while tanh(A)/tanh(B) of the
            # previous step complete.  h-slot order gives each tanh half
            # ~10 slots of downstream fill before its next-step consumers.
            from concourse.tile import add_dep_helper
            rnn_ps = {}

            def emit_u(t, after=None):
                uc = (t % CHUNK) * BL
                urhs = u_tiles[t // CHUNK][:, uc:uc + BL]
                ps_a = rnnps.tile([128, 2 * BL], f32, name="ps_a")
                ps_b = rnnps.tile([128, 2 * BL], f32, name="ps_b")
                rnn_ps[t] = (ps_a, ps_b)
                for m in range(4):
                    o = (ps_a, ps_a, ps_b, ps_b)[m][:, BL * (m % 2):
                                                    BL * (m % 2 + 1)]
                    inst = mm(o, wu_sb[:, 128 * m:128 * (m + 1)], urhs,
                              start=(m % 2 == 0), stop=(t == 0),
                              skip_group_check=True)
                    if after is not None:
                        add_dep_helper(inst.ins, after.ins, sync=False,
                                       reason="pin u-fill to period tail")

            # ---- x_dnn (state MLP) work queue, drained into the idle PE/ACT
            # slots of RNN steps >= XDNN_T0: items are ('mm', fn) emitted
            # after a step's u-fill, or ('act', fn) emitted right after a
            # step's tanh(B) where the ACT engine has ~480ns of idle.
            # Every instruction is order-pinned (add_dep_helper) to the step
            # it is drained at: Tile's list scheduler otherwise hoists these
            # to ~22us (as soon as the x-weight DMAs land), which stretches
            # the early RNN periods, drops PE duty, and triggers a ~13us
            # HAM re-throttle window. ----
            xwork = []
            xst = {}

            def _pin(inst, after):
                if after is not None and inst is not None:
                    add_dep_helper(inst.ins, after.ins, sync=False,
                                   reason="pin x-work to its drain step")
                return inst

            def _xl1_mms(half):
                def f(after):
                    p = xst.setdefault(f"p1{half}", mlptile())
                    in_blocks = [xt_sb[:, 0:BL], xt_sb[:, BL:2 * BL]]
                    for mi in range(4):
                        m = half * 4 + mi
                        o = p[:, BL * mi:BL * (mi + 1)]
                        _pin(mm(o, x1tb_sb[:, 128 * m:128 * (m + 1)],
                                tb_sb[:, :], start=(mi == 0), stop=False,
                                skip_group_check=True), after)
                        for k in range(2):
                            j = k * 8 + m
                            _pin(mm(o, x1_sb[:, 128 * j:128 * (j + 1)],
                                    in_blocks[k], start=False, stop=(k == 1),
                                    skip_group_check=True), after)
                return f

            def _xact(src_key, dst_key, dst_shape, q):
                def f(after):
                    dst = xst.setdefault(dst_key,
                                         work.tile([128, dst_shape], bf,
                                                   name=dst_key))
                    _pin(nc.scalar.activation(
                        dst[:, q * 2 * BL:(q + 1) * 2 * BL],
                        xst[src_key][:, (q % 2) * 2 * BL:
                                     (q % 2 + 1) * 2 * BL], TANH), after)
                return f

            def _xl2_mms(half, mi):
                def f(after):
                    p = xst.setdefault(f"p2{half}", mlptile())
                    m = half * 4 + mi
                    o = p[:, BL * mi:BL * (mi + 1)]
                    _pin(mm(o, x2b_sb[:, 128 * m:128 * (m + 1)],
                            ones_sb[:, :], start=(mi == 0), stop=False,
                            skip_group_check=True), after)
                    for k in range(8):
                        j = k * 8 + m
                        _pin(mm(o, x2_sb[:, 128 * j:128 * (j + 1)],
                                xst["xa1"][:, BL * k:BL * (k + 1)],
                                start=False, stop=(k == 7),
                                skip_group_check=True), after)
                return f

            def _xl3_mms(m):
                def f(after):
                    p = xst.setdefault("p3", mlptile())
                    o = p[:, BL * m:BL * (m + 1)]
                    _pin(mm(o, x3b_sb[:, 128 * m:128 * (m + 1)],
                            ones_sb[:, :], start=(m == 0), stop=False,
                            skip_group_check=True), after)
                    for k in range(8):
                        j = k * 2 + m
                        _pin(mm(o, x3_sb[:, 128 * j:128 * (j + 1)],
                                xst["xa2"][:, BL * k:BL * (k + 1)],
                                start=False, stop=(k == 7),
                                skip_group_check=True), after)
                return f

            def _xcopy(after):
                s_t = work.tile([128, 2 * BL], bf, name="s_sb")
                xst["s_sb"] = s_t
                nc.vector.tensor_copy(s_t[:, :], xst["p3"][:, 0:2 * BL])

            for half in (0, 1):
                xwork.append(('mm', _xl1_mms(half)))        # 12 mms each
            for half in (0, 1):
                for q in (0, 1):
                    xwork.append(('act', _xact(f"p1{half}", "xa1", 8 * BL,
                                               half * 2 + q)))
            for half in (0, 1):
                for mi in range(4):
                    xwork.append(('mm', _xl2_mms(half, mi)))  # 9 mms each
            for half in (0, 1):
                for q in (0, 1):
                    xwork.append(('act', _xact(f"p2{half}", "xa2", 8 * BL,
                                               half * 2 + q)))
            for m in (0, 1):
                xwork.append(('mm', _xl3_mms(m)))             # 9 mms each
            xwork.append(('mm', _xcopy))

            u_tiles = {0: u_tile}
            emit_u(0)
            emit_u(1)
            hcur = None
            for t in range(NSTEPS):
                tpre = t + 8
                if tpre % CHUNK == 0 and tpre <= NSTEPS - 1:
                    nt = upool.tile([CD + 1, CHUNK * BL], bf, name="ut")
                    nc.sync.dma_start(
                        out=nt[:, :],
                        in_=u_aug_d[:, tpre * BL:(tpre + CHUNK) * BL])
                    u_tiles[tpre // CHUNK] = nt
                    u_tiles.pop(tpre // CHUNK - 2, None)
                ps_a, ps_b = rnn_ps.pop(t)
                psb = (ps_a, ps_a, ps_b, ps_b)

                def reg(m):
                    return psb[m][:, BL * (m % 2):BL * (m % 2 + 1)]

                # allocate in the iteration that writes it: a bottom-of-
                # previous-iteration alloc defeats Tile's release-join
                # scoping (the 'h_hpool release without same-scope alloc'
                # validation warning) and costs a conservative extra sem
                # on the gating tanh
                hnew = hpool.tile([128, 4 * BL], bf, name="h")
                last_h = None
                if t > 0:
                    def hmm(m, k):
                        return mm(reg(m), wh_sb[:, 128 * (k * 4 + m):
                                                128 * (k * 4 + m + 1)],
                                  hcur[:, BL * k:BL * (k + 1)],
                                  start=False, stop=(k == 3),
                                  skip_group_check=True)
                    # slots: k01A(4) k01B(2) k23A(4) [tanh A]
                    #        k01B(2) k23B(4) [tanh B]  u(t+2) x4
                    for m, k in ((0, 0), (1, 0), (0, 1), (1, 1),
                                 (2, 0), (3, 0),
                                 (0, 2), (0, 3), (1, 2), (1, 3)):
                        hmm(m, k)
                    nc.scalar.activation(hnew[:, 0:2 * BL], ps_a[:, :], TANH)
                    for m, k in ((2, 1), (3, 1),
                                 (2, 2), (2, 3), (3, 2), (3, 3)):
                        last_h = hmm(m, k)
                else:
                    nc.scalar.activation(hnew[:, 0:2 * BL], ps_a[:, :], TANH)
                tb_inst = nc.scalar.activation(hnew[:, 2 * BL:4 * BL],
                                               ps_b[:, :], TANH)
                xdrain = (t >= XDNN_T0 and (t - XDNN_T0) % XDNN_EVERY == 0)
                if xdrain and xwork and xwork[0][0] == 'act':
                    xwork.pop(0)[1](tb_inst)
                    xdrain = False
                tn = t + 2
                if tn <= NSTEPS - 1:
                    emit_u(tn, after=last_h)
                if xdrain and xwork and xwork[0][0] == 'mm':
                    xwork.pop(0)[1](last_h)
                # dense dummy fill through the first steps (`dummy` is
                # memset to 0, so accumulating dummy@dummy into the (t+2)
                # pre-activation banks is numerically a no-op): the HAM MID
                # window re-throttles the PE clock if duty drops right
                # after the warmup ends, before the steady pattern settles.
                # Not worth running for all t: Tile occasionally schedules
                # a fill right before the tanh gate it feeds, stretching
                # that period by ~1.3us.
                if last_h is not None and 1 <= t <= 10:
                    fa, fb = rnn_ps[tn]
                    for i in range(12):
                        fi = mm((fa if i % 2 == 0 else fb)[:, 0:BL],
                                dummy[:, :], dummy[:, 0:BL],
                                start=False, stop=False,
                                skip_group_check=True)
                        add_dep_helper(fi.ins, last_h.ins, sync=False,
                                       reason="early HAM fill")
                if t in delayed_loads:
                    for go in delayed_loads[t]:
                        go(last_h)
                hcur = hnew
                if debug and t in (0, 1):
                    nc.gpsimd.dma_start(out=dbg[f"dbg_h{t}"][:, :],
                                        in_=hcur[:, :])
            if debug:
                nc.gpsimd.dma_start(out=dbg["dbg_hlast"][:, :], in_=hcur[:, :])
            rnnps_ctx.__exit__(None, None, None)
            # tail pool in the 12KB the RNN PSUM pool just freed: enough
            # buffers that no layer's first matmul waits on a previous
            # layer's tanh to release its tile
            tail_ctx = tc.tile_pool(name="tailps", bufs=6, space="PSUM")
            tailps = tail_ctx.__enter__()

            def mlptile():
                return tailps.tile([128, 4 * BL], f32, name="tmlp")

            fps = tailps.tile([128, 64], f32, name="tmlp")  # shares slot ring

            def pefill(n):
                for _ in range(n):
                    mm(fps[:, :], dummy[:, :], dummy[:, 0:64],
                       start=True, stop=True, skip_group_check=True)

            # ---------------- h2o: r = tanh(h2o_W @ [u_last; h_last] + b) ----
            uc_last = ((T - 1) % CHUNK) * BL
            u_last_tile = u_tiles[(T - 1) // CHUNK]
            ps = mlptile()
            for m in range(2):
                mm(ps[:, BL * m:BL * (m + 1)],
                   h2ou_sb[:, 128 * m:128 * (m + 1)],
                   u_last_tile[:, uc_last:uc_last + BL], start=(m == 0),
                   stop=False, skip_group_check=True)
                for k in range(4):
                    j = k * 2 + m
                    mm(ps[:, BL * m:BL * (m + 1)],
                       h2o_sb[:, 128 * j:128 * (j + 1)],
                       hcur[:, BL * k:BL * (k + 1)],
                       start=False, stop=(k == 3), skip_group_check=True)
            r_sb = work.tile([128, 2 * BL], bf, name="r_sb")

            # ---------------- u_dnn tail --------------------------------
            # One wide ACT per PSUM tile (tail ACTs self-serialize at
            # ~400ns each, so fewer/wider beats 4x128-col), bias matmuls
            # emitted first (dep-free, they fill the tanh-latency windows),
            # and k-groups emitted per ACT half so layer N+1's matmuls
            # stream while layer N's second tanh is still running.
            nc.scalar.activation(r_sb[:, 0:2 * BL], ps[:, 0:2 * BL], TANH)
            pefill(12)

            ups1 = [mlptile(), mlptile()]
            for half in range(2):
                for mi in range(4):
                    m = half * 4 + mi
                    mm(ups1[half][:, BL * mi:BL * (mi + 1)],
                       u1tb_sb[:, 128 * m:128 * (m + 1)], tb_sb[:, :],
                       start=(mi == 0), stop=False, skip_group_check=True)
            for k in range(2):
                for half in range(2):
                    for mi in range(4):
                        m = half * 4 + mi
                        j = k * 8 + m
                        mm(ups1[half][:, BL * mi:BL * (mi + 1)],
                           u1_sb[:, 128 * j:128 * (j + 1)],
                           r_sb[:, BL * k:BL * (k + 1)],
                           start=False, stop=(k == 1), skip_group_check=True)
            ua1 = work.tile([128, 8 * BL], bf, name="ua1")
            nc.scalar.activation(ua1[:, 0:4 * BL], ups1[0][:, :], TANH)
            nc.scalar.activation(ua1[:, 4 * BL:8 * BL], ups1[1][:, :], TANH)
            pefill(8)

            ups2 = [mlptile(), mlptile()]
            for half in range(2):
                for mi in range(4):
                    m = half * 4 + mi
                    mm(ups2[half][:, BL * mi:BL * (mi + 1)],
                       u2b_sb[:, 128 * m:128 * (m + 1)], ones_sb[:, :],
                       start=(mi == 0), stop=False, skip_group_check=True)
            for k in range(8):
                for half in range(2):
                    for mi in range(4):
                        m = half * 4 + mi
                        j = k * 8 + m
                        mm(ups2[half][:, BL * mi:BL * (mi + 1)],
                           u2_sb[:, 128 * j:128 * (j + 1)],
                           ua1[:, BL * k:BL * (k + 1)],
                           start=False, stop=(k == 7), skip_group_check=True)
            ua2 = work.tile([128, 8 * BL], bf, name="ua2")
            nc.scalar.activation(ua2[:, 0:4 * BL], ups2[0][:, :], TANH)
            nc.scalar.activation(ua2[:, 4 * BL:8 * BL], ups2[1][:, :], TANH)
            pefill(10)

            ups3 = mlptile()
            for m in range(2):
                mm(ups3[:, BL * m:BL * (m + 1)],
                   u3b_sb[:, 128 * m:128 * (m + 1)], ones_sb[:, :],
                   start=(m == 0), stop=False, skip_group_check=True)
            for k in range(8):
                for m in range(2):
                    j = k * 2 + m
                    mm(ups3[:, BL * m:BL * (m + 1)],
                       u3_sb[:, 128 * j:128 * (j + 1)],
                       ua2[:, BL * k:BL * (k + 1)],
                       start=False, stop=(k == 7), skip_group_check=True)
            s_sb = xst["s_sb"]
            c_sb = work.tile([128, 2 * BL], bf, name="c_sb")
            nc.vector.tensor_copy(c_sb[:, :], ups3[:, 0:2 * BL])

            # ---------------- combinator ----------------
            # bias + state-part matmuls are c_sb-independent: they accumulate
            # while the ups3 -> c_sb copy is still in flight, leaving only
            # the 4 control-part matmuls on the final serial chain
            ps = mlptile()
            for m in range(2):
                o = ps[:, BL * m:BL * (m + 1)]
                mm(o, combb_sb[:, 128 * m:128 * (m + 1)], ones_sb[:, :],
                   start=(m == 0), stop=False, skip_group_check=True)
                for k in range(2):
                    j = k * 2 + m
                    mm(o, comb_sb[:, 128 * j:128 * (j + 1)],
                       s_sb[:, BL * k:BL * (k + 1)],
                       start=False, stop=False, skip_group_check=True)
            pefill(10)
            for m in range(2):
                o = ps[:, BL * m:BL * (m + 1)]
                for k in range(2, 4):
                    j = k * 2 + m
                    mm(o, comb_sb[:, 128 * j:128 * (j + 1)],
                       c_sb[:, BL * (k - 2):BL * (k - 1)],
                       start=False, stop=(k == 3), skip_group_check=True)
            out_sb = work.tile([128, 2 * BL], f32, name="out_sb")
            nc.vector.tensor_copy(out_sb[:, :], ps[:, 0:2 * BL])
            # two queues so the two halves of the output DMA overlap
            nc.sync.dma_start(out=out_d[0:128, :], in_=out_sb[:, 0:BL])
            nc.gpsimd.dma_start(out=out_d[128:256, :], in_=out_sb[:, BL:2 * BL])
            tail_ctx.__exit__(None, None, None)
            mlpps_ctx.__exit__(None, None, None)

    nc.compile()
    return nc


def _get_program():
    if "nc" not in _CACHE:
        _CACHE["nc"] = _build_program()
    return _CACHE["nc"]


def run(inputs, trace=False, trace_cores=None):
    from concourse.bass_utils import run_bass_kernel_spmd

    nc = _get_program()
    w = _weight_arrays(inputs)
    in_maps = []
    for c in range(NCORES):
        m = dict(w)
        m.update(_per_core_arrays(inputs, c))
        in_maps.append(m)
    res = run_bass_kernel_spmd(nc, in_maps, list(range(NCORES)),
                               trace=trace, trace_cores=trace_cores)
    out = np.empty((B, SD), np.float32)
    for c in range(NCORES):
        out[c * BL:(c + 1) * BL, :] = np.asarray(res.results[c]["out"]).T
    return out, res


def kernel(**inputs):
    out, _ = run(inputs)
    return out



# revision 38
# speedup vs baseline: 1.0092x; 1.0092x over previous
"""Trainium2 Bass kernel for CausalFlowModel (RNN scan + 2 MLPs + combinator).

Sharding: data-parallel over batch across 8 NeuronCores (64 rows/core).
All weights replicated, pre-transposed+packed on host into lhsT tile banks.
Everything on-device runs in bf16 with fp32 PSUM accumulation; biases are
folded into the matmuls via an appended ones-row (they are all zero for this
problem, but handled correctly anyway).

Layout convention: all activations live TRANSPOSED in SBUF as
[feature-partition, batch-column] so the 511-step recurrence needs no
per-step transposes:  hT_{t+1}[m-block] = tanh( sum_k WhT[k,m].T @ hT_t[k]
                                               + WuT_aug[m].T @ uT_aug_t )

RNN step schedule (the performance-critical part): each step's pre-activation
accumulates into TWO PSUM banks (bank A = h-regions 0..1, bank B = 2..3) so
the two tanh halves on ScalarE can each overlap TensorE work on the OTHER
bank.  The measured steady-state period (1072ns) sits ~30ns above the
structural floor 52(sem) + 115(4 gated issues) + 194(drain) + 90(sem) +
265(ScalarE ACT issue spacing) + 355(ACT dur): both tanhs serialize on
ScalarE and every bank needs both k-halves, so the floor is invariant to
slot order / bank split.  Slot order per step:
    u0(A) u1(A) u2(B) u3(B)                # independent of h, fills latency
    (m01,k01 -> A) (m23,k01 -> B)          # consume prev blocks 0,1 only
    (m01,k23 -> A)  [tanh A]  (m23,k23 -> B)  [tanh B]
Scheduling hygiene matters as much as the period: x_dnn work is order-pinned
to its drain step (Tile otherwise hoists it into the early RNN), the big
MLP-weight DMAs are sem-pinned to mid-RNN steps (their transfers otherwise
collide with the early RNN's SBUF traffic and trip the HAM re-throttle), and
dummy-matmul fills keep PE duty high across the warmup->RNN and RNN->tail
transitions.
"""

import numpy as np
import ml_dtypes

B, T = 512, 512
SD, CD, H = 256, 64, 512
D1, D2 = 1024, 1024
NCORES = 8
BL = B // NCORES          # 64 batch rows per core
CHUNK = 64                # u steps per DMA chunk
WARMUP_MM = 130           # dense dummy matmuls before the RNN; HAM flip to
                          # 2.4GHz measured ~4.5us (~85 cold MMs) after the
                          # first MM, worst-case phase needs ~6.8us (~128)
XDNN_T0 = 160             # RNN step where interleaved x_dnn work begins
XDNN_EVERY = 4            # drain one x_dnn item per this many steps: keeps
                          # the extra ScalarE/PE load too thin to disturb
                          # the HAM activity window
NSTEPS = T - 1            # 511 scan steps

_BF = ml_dtypes.bfloat16

_CACHE = {}


def _bf16(a):
    return np.ascontiguousarray(np.asarray(a, np.float32)).astype(_BF)


def _pack_kxm(W, n_m, n_k, k_off=0):
    """lhsT tile bank [128, n_k*n_m*128]; block j=k*n_m+m is
    W[m*128:(m+1)*128, k_off+k*128 : k_off+(k+1)*128].T"""
    cols = []
    for k in range(n_k):
        for m in range(n_m):
            cols.append(W[m * 128:(m + 1) * 128,
                          k_off + k * 128: k_off + (k + 1) * 128].T)
    return np.concatenate(cols, axis=1)


def _pack_head_bias(W, bvec, n_m, width):
    """[width+1, n_m*128]; block m = [W[m*128:(m+1)*128, :width].T ; b[mblock]]"""
    cols = []
    for m in range(n_m):
        blk = np.concatenate(
            [W[m * 128:(m + 1) * 128, :width].T,
             bvec[m * 128:(m + 1) * 128][None, :]], axis=0)
        cols.append(blk)
    return np.concatenate(cols, axis=1)


def _weight_arrays(inp):
    i2h_W, i2h_b = inp["i2h_W"], inp["i2h_b"]
    w = {
        "whT": _pack_kxm(i2h_W, 4, 4, k_off=CD),
        "wuT": _pack_head_bias(i2h_W, i2h_b, 4, CD),          # [65, 512]
        "x1T": _pack_kxm(inp["x1_W"], 8, 2, k_off=1),
        "x1tb": _pack_head_bias(inp["x1_W"], inp["x1_b"], 8, 1),  # [2, 1024]
        "x2T": _pack_kxm(inp["x2_W"], 8, 8),
        "x2b": np.asarray(inp["x2_b"], np.float32)[None, :],
        "x3T": _pack_kxm(inp["x3_W"], 2, 8),
        "x3b": np.asarray(inp["x3_b"], np.float32)[None, :],
        "u1T": _pack_kxm(inp["u1_W"], 8, 2, k_off=1),
        "u1tb": _pack_head_bias(inp["u1_W"], inp["u1_b"], 8, 1),
        "u2T": _pack_kxm(inp["u2_W"], 8, 8),
        "u2b": np.asarray(inp["u2_b"], np.float32)[None, :],
        "u3T": _pack_kxm(inp["u3_W"], 2, 8),
        "u3b": np.asarray(inp["u3_b"], np.float32)[None, :],
        "h2oT": _pack_kxm(inp["h2o_W"], 2, 4, k_off=CD),
        "h2o_uT": _pack_head_bias(inp["h2o_W"], inp["h2o_b"], 2, CD),  # [65, 256]
        "combT": _pack_kxm(inp["comb_W"], 2, 4),
        "combb": np.asarray(inp["comb_b"], np.float32)[None, :],
    }
    return {k: _bf16(v) for k, v in w.items()}


def _per_core_arrays(inp, c):
    t = np.asarray(inp["t"], np.float32)
    x = np.asarray(inp["x"], np.float32)
    u = np.asarray(inp["u"], np.float32)
    b0 = c * BL
    us = u[:, b0:b0 + BL, :].transpose(2, 0, 1).reshape(CD, T * BL)
    u_aug = np.concatenate([us, np.ones((1, T * BL), np.float32)], axis=0)
    xT = x[b0:b0 + BL].T                              # [256, BL]
    xt = np.concatenate([xT[:128], xT[128:]], axis=1)  # [128, 2*BL]
    tb = np.stack([t[b0:b0 + BL, 0], np.ones(BL, np.float32)], axis=0)  # [2, BL]
    return {"u_aug": _bf16(u_aug), "xt": _bf16(xt), "tb": _bf16(tb)}


def _build_program(debug=False):
    import concourse.bass as bass
    import concourse.mybir as mybir
    from concourse import bacc
    from concourse.tile import TileContext

    bf = mybir.dt.bfloat16
    f32 = mybir.dt.float32
    TANH = mybir.ActivationFunctionType.Tanh

    nc = bacc.Bacc("TRN2", target_bir_lowering=False, debug=False)

    d_in = {}
    def din(name, shape, dt=bf):
        d_in[name] = nc.dram_tensor(name, list(shape), dt, kind="ExternalInput")
        return d_in[name]

    u_aug_d = din("u_aug", (CD + 1, T * BL))
    xt_d = din("xt", (128, 2 * BL))
    tb_d = din("tb", (2, BL))
    wh_d = din("whT", (128, 16 * 128))
    wu_d = din("wuT", (CD + 1, 4 * 128))
    x1_d = din("x1T", (128, 16 * 128))
    x1tb_d = din("x1tb", (2, 8 * 128))
    x2_d = din("x2T", (128, 64 * 128))
    x2b_d = din("x2b", (1, 8 * 128))
    x3_d = din("x3T", (128, 16 * 128))
    x3b_d = din("x3b", (1, 2 * 128))
    u1_d = din("u1T", (128, 16 * 128))
    u1tb_d = din("u1tb", (2, 8 * 128))
    u2_d = din("u2T", (128, 64 * 128))
    u2b_d = din("u2b", (1, 8 * 128))
    u3_d = din("u3T", (128, 16 * 128))
    u3b_d = din("u3b", (1, 2 * 128))
    h2o_d = din("h2oT", (128, 8 * 128))
    h2ou_d = din("h2o_uT", (CD + 1, 2 * 128))
    comb_d = din("combT", (128, 8 * 128))
    combb_d = din("combb", (1, 2 * 128))
    out_d = nc.dram_tensor("out", [2 * 128, BL], f32, kind="ExternalOutput")
    dbg = {}
    if debug:
        for name in ("dbg_h0", "dbg_h1", "dbg_hlast"):
            dbg[name] = nc.dram_tensor(name, [128, 4 * BL], f32,
                                       kind="ExternalOutput")
        for name in ("dbg_r", "dbg_s", "dbg_c"):
            dbg[name] = nc.dram_tensor(name, [128, 2 * BL], f32,
                                       kind="ExternalOutput")

    with TileContext(nc) as tc:
        with (
            tc.tile_pool(name="consts", bufs=1) as consts,
            tc.tile_pool(name="upool", bufs=2) as upool,
            tc.tile_pool(name="hpool", bufs=6) as hpool,
            tc.tile_pool(name="work", bufs=1) as work,
        ):
            mm = nc.tensor.matmul
            # --- PE p-state warmup: dense dummy stream overlapping the ---
            # --- initial DMA wait; pushes the PE clock to 2.4GHz early ---
            warm_ctx = tc.tile_pool(name="warmps", bufs=1, space="PSUM")
            warmps = warm_ctx.__enter__()
            dummy = work.tile([128, 128], bf, name="dummy")
            nc.vector.memset(dummy[:, :], 0.0)
            wps = warmps.tile([128, 64], f32, name="wps")
            for _ in range(WARMUP_MM):
                mm(wps[:, :], dummy[:, :], dummy[:, 0:64],
                   start=True, stop=True, skip_group_check=True)

            def cload(dram, shape, dt=bf, name=None):
                tile = consts.tile(list(shape), dt, name=name)
                nc.sync.dma_start(out=tile[:, :], in_=dram[:, :])
                return tile

            # --- DMAs the RNN needs first, ordered so the scan starts ASAP:
            # a small head of u-chunk 0, then wu, then wh k-slices, then the
            # rest of chunk 0 ---
            u_tile = upool.tile([CD + 1, CHUNK * BL], bf, name="ut")
            head = 8 * BL
            nc.sync.dma_start(out=u_tile[:, 0:head], in_=u_aug_d[:, 0:head])
            wu_sb = cload(wu_d, (CD + 1, 4 * 128), name="wu_sb")
            wh_sb = consts.tile([128, 16 * 128], bf, name="wh_sb")
            for kk in range(4):
                nc.sync.dma_start(out=wh_sb[:, kk * 512:(kk + 1) * 512],
                                  in_=wh_d[:, kk * 512:(kk + 1) * 512])
            nc.sync.dma_start(out=u_tile[:, head:CHUNK * BL],
                              in_=u_aug_d[:, head:CHUNK * BL])
            # --- remaining consts (stream in during the RNN) ---
            tb_sb = cload(tb_d, (2, BL), name="tb_sb")
            ones_sb = consts.tile([1, BL], bf, name="ones_sb")
            nc.sync.dma_start(out=ones_sb[:, :], in_=tb_d[1:2, :])
            xt_sb = cload(xt_d, (128, 2 * BL), name="xt_sb")
            h2o_sb = cload(h2o_d, (128, 8 * 128), name="h2o_sb")
            h2ou_sb = cload(h2ou_d, (CD + 1, 2 * 128), name="h2ou_sb")

            # The MLP weights (~7.5MB, first needed at step XDNN_T0) are
            # DMAed mid-RNN, sem-pinned to step milestones: streaming them
            # at kernel start collides with the early RNN's SBUF traffic
            # (observed ~2us TE stalls -> HAM re-throttle).
            def dload(dram, shape, name):
                tile = consts.tile(list(shape), bf, name=name)

                def go(after):
                    inst = nc.sync.dma_start(out=tile[:, :], in_=dram[:, :])
                    if after is not None:
                        add_dep_helper(inst.ins, after.ins, sync=True,
                                       reason="delay big const DMA")
                return tile, go

            x1_sb, x1_go = dload(x1_d, (128, 16 * 128), name="x1_sb")
            x1tb_sb, x1tb_go = dload(x1tb_d, (2, 8 * 128), name="x1tb_sb")
            x2_sb, x2_go = dload(x2_d, (128, 64 * 128), name="x2_sb")
            x2b_sb, x2b_go = dload(x2b_d, (1, 8 * 128), name="x2b_sb")
            x3_sb, x3_go = dload(x3_d, (128, 16 * 128), name="x3_sb")
            x3b_sb, x3b_go = dload(x3b_d, (1, 2 * 128), name="x3b_sb")
            u1_sb, u1_go = dload(u1_d, (128, 16 * 128), name="u1_sb")
            u1tb_sb, u1tb_go = dload(u1tb_d, (2, 8 * 128), name="u1tb_sb")
            u2_sb, u2_go = dload(u2_d, (128, 64 * 128), name="u2_sb")
            u2b_sb, u2b_go = dload(u2b_d, (1, 8 * 128), name="u2b_sb")
            u3_sb, u3_go = dload(u3_d, (128, 16 * 128), name="u3_sb")
            u3b_sb, u3b_go = dload(u3b_d, (1, 2 * 128), name="u3b_sb")
            comb_sb, comb_go = dload(comb_d, (128, 8 * 128), name="comb_sb")
            combb_sb, combb_go = dload(combb_d, (1, 2 * 128), name="combb_sb")
            delayed_loads = {
                60: [x1_go, x1tb_go], 100: [x2_go], 150: [x2b_go, x3_go],
                190: [x3b_go, u1_go, u1tb_go], 230: [u2_go],
                280: [u2b_go, u3_go], 320: [u3b_go, comb_go, combb_go],
            }

            warm_ctx.__exit__(None, None, None)
            mlpps_ctx = tc.tile_pool(name="mlpps", bufs=2, space="PSUM")
            mlpps = mlpps_ctx.__enter__()


            def mlptile():
                return mlpps.tile([128, 4 * BL], f32, name="mlp")

            rnnps_ctx = tc.tile_pool(name="rnnps", bufs=3, space="PSUM")
            rnnps = rnnps_ctx.__enter__()

            # ---------------- RNN scan: 511 steps ----------------
            # The u-part matmuls for step t+2 are emitted at the END of
            # iteration t (explicit 2-deep software pipeline): they are the
            # only h-independent PE work, and placing them right after each
            # step's tail keeps the PE busy while tanh(A)/tanh(B) of the
            # previous step complete.  h-slot order gives each tanh half
            # ~10 slots of downstream fill before its next-step consumers.
            from concourse.tile import add_dep_helper
            rnn_ps = {}

            def emit_u(t, after=None):
                uc = (t % CHUNK) * BL
                urhs = u_tiles[t // CHUNK][:, uc:uc + BL]
                ps_a = rnnps.tile([128, 2 * BL], f32, name="ps_a")
                ps_b = rnnps.tile([128, 2 * BL], f32, name="ps_b")
                rnn_ps[t] = (ps_a, ps_b)
                for m in range(4):
                    o = (ps_a, ps_a, ps_b, ps_b)[m][:, BL * (m % 2):
                                                    BL * (m % 2 + 1)]
                    inst = mm(o, wu_sb[:, 128 * m:128 * (m + 1)], urhs,
                              start=(m % 2 == 0), stop=(t == 0),
                              skip_group_check=True)
                    if after is not None:
                        add_dep_helper(inst.ins, after.ins, sync=False,
                                       reason="pin u-fill to period tail")

            # ---- x_dnn (state MLP) work queue, drained into the idle PE/ACT
            # slots of RNN steps >= XDNN_T0: items are ('mm', fn) emitted
            # after a step's u-fill, or ('act', fn) emitted right after a
            # step's tanh(B) where the ACT engine has ~480ns of idle.
            # Every instruction is order-pinned (add_dep_helper) to the step
            # it is drained at: Tile's list scheduler otherwise hoists these
            # to ~22us (as soon as the x-weight DMAs land), which stretches
            # the early RNN periods, drops PE duty, and triggers a ~13us
            # HAM re-throttle window. ----
            xwork = []
            xst = {}

            def _pin(inst, after):
                if after is not None and inst is not None:
                    add_dep_helper(inst.ins, after.ins, sync=False,
                                   reason="pin x-work to its drain step")
                return inst

            def _xl1_mms(half):
                def f(after):
                    p = xst.setdefault(f"p1{half}", mlptile())
                    in_blocks = [xt_sb[:, 0:BL], xt_sb[:, BL:2 * BL]]
                    for mi in range(4):
                        m = half * 4 + mi
                        o = p[:, BL * mi:BL * (mi + 1)]
                        _pin(mm(o, x1tb_sb[:, 128 * m:128 * (m + 1)],
                                tb_sb[:, :], start=(mi == 0), stop=False,
                                skip_group_check=True), after)
                        for k in range(2):
                            j = k * 8 + m
                            _pin(mm(o, x1_sb[:, 128 * j:128 * (j + 1)],
                                    in_blocks[k], start=False, stop=(k == 1),
                                    skip_group_check=True), after)
                return f

            def _xact(src_key, dst_key, dst_shape, q):
                def f(after):
                    dst = xst.setdefault(dst_key,
                                         work.tile([128, dst_shape], bf,
                                                   name=dst_key))
                    _pin(nc.scalar.activation(
                        dst[:, q * 2 * BL:(q + 1) * 2 * BL],
                        xst[src_key][:, (q % 2) * 2 * BL:
                                     (q % 2 + 1) * 2 * BL], TANH), after)
                return f

            def _xl2_mms(half, mi):
                def f(after):
                    p = xst.setdefault(f"p2{half}", mlptile())
                    m = half * 4 + mi
                    o = p[:, BL * mi:BL * (mi + 1)]
                    _pin(mm(o, x2b_sb[:, 128 * m:128 * (m + 1)],
                            ones_sb[:, :], start=(mi == 0), stop=False,
                            skip_group_check=True), after)
                    for k in range(8):
                        j = k * 8 + m
                        _pin(mm(o, x2_sb[:, 128 * j:128 * (j + 1)],
                                xst["xa1"][:, BL * k:BL * (k + 1)],
                                start=False, stop=(k == 7),
                                skip_group_check=True), after)
                return f

            def _xl3_mms(m):
                def f(after):
                    p = xst.setdefault("p3", mlptile())
                    o = p[:, BL * m:BL * (m + 1)]
                    _pin(mm(o, x3b_sb[:, 128 * m:128 * (m + 1)],
                            ones_sb[:, :], start=(m == 0), stop=False,
                            skip_group_check=True), after)
                    for k in range(8):
                        j = k * 2 + m
                        _pin(mm(o, x3_sb[:, 128 * j:128 * (j + 1)],
                                xst["xa2"][:, BL * k:BL * (k + 1)],
                                start=False, stop=(k == 7),
                                skip_group_check=True), after)
                return f

            def _xcopy(after):
                s_t = work.tile([128, 2 * BL], bf, name="s_sb")
                xst["s_sb"] = s_t
                nc.vector.tensor_copy(s_t[:, :], xst["p3"][:, 0:2 * BL])

            for half in (0, 1):
                xwork.append(('mm', _xl1_mms(half)))        # 12 mms each
            for half in (0, 1):
                for q in (0, 1):
                    xwork.append(('act', _xact(f"p1{half}", "xa1", 8 * BL,
                                               half * 2 + q)))
            for half in (0, 1):
                for mi in range(4):
                    xwork.append(('mm', _xl2_mms(half, mi)))  # 9 mms each
            for half in (0, 1):
                for q in (0, 1):
                    xwork.append(('act', _xact(f"p2{half}", "xa2", 8 * BL,
                                               half * 2 + q)))
            for m in (0, 1):
                xwork.append(('mm', _xl3_mms(m)))             # 9 mms each
            xwork.append(('mm', _xcopy))

            u_tiles = {0: u_tile}
            emit_u(0)
            emit_u(1)
            hcur = None
            for t in range(NSTEPS):
                tpre = t + 8
                if tpre % CHUNK == 0 and tpre <= NSTEPS - 1:
                    nt = upool.tile([CD + 1, CHUNK * BL], bf, name="ut")
                    nc.sync.dma_start(
                        out=nt[:, :],
                        in_=u_aug_d[:, tpre * BL:(tpre + CHUNK) * BL])
                    u_tiles[tpre // CHUNK] = nt
                    u_tiles.pop(tpre // CHUNK - 2, None)
                ps_a, ps_b = rnn_ps.pop(t)
                psb = (ps_a, ps_a, ps_b, ps_b)

                def reg(m):
                    return psb[m][:, BL * (m % 2):BL * (m % 2 + 1)]

                # allocate in the iteration that writes it: a bottom-of-
                # previous-iteration alloc defeats Tile's release-join
                # scoping (the 'h_hpool release without same-scope alloc'
                # validation warning) and costs a conservative extra sem
                # on the gating tanh
                hnew = hpool.tile([128, 4 * BL], bf, name="h")
                last_h = None
                if t > 0:
                    def hmm(m, k):
                        return mm(reg(m), wh_sb[:, 128 * (k * 4 + m):
                                                128 * (k * 4 + m + 1)],
                                  hcur[:, BL * k:BL * (k + 1)],
                                  start=False, stop=(k == 3),
                                  skip_group_check=True)
                    # slots: k01A(4) k01B(2) k23A(4) [tanh A]
                    #        k01B(2) k23B(4) [tanh B]  u(t+2) x4
                    for m, k in ((0, 0), (1, 0), (0, 1), (1, 1),
                                 (2, 0), (3, 0),
                                 (0, 2), (0, 3), (1, 2), (1, 3)):
                        hmm(m, k)
                    nc.scalar.activation(hnew[:, 0:2 * BL], ps_a[:, :], TANH)
                    for m, k in ((2, 1), (3, 1),
                                 (2, 2), (2, 3), (3, 2), (3, 3)):
                        last_h = hmm(m, k)
                else:
                    nc.scalar.activation(hnew[:, 0:2 * BL], ps_a[:, :], TANH)
                tb_inst = nc.scalar.activation(hnew[:, 2 * BL:4 * BL],
                                               ps_b[:, :], TANH)
                xdrain = (t >= XDNN_T0 and (t - XDNN_T0) % XDNN_EVERY == 0)
                if xdrain and xwork and xwork[0][0] == 'act':
                    xwork.pop(0)[1](tb_inst)
                    xdrain = False
                tn = t + 2
                if tn <= NSTEPS - 1:
                    emit_u(tn, after=last_h)
                if xdrain and xwork and xwork[0][0] == 'mm':
                    xwork.pop(0)[1](last_h)
                # dense dummy fill through the first steps (`dummy` is
                # memset to 0, so accumulating dummy@dummy into the (t+2)
                # pre-activation banks is numerically a no-op): the HAM MID
                # window re-throttles the PE clock if duty drops right
                # after the warmup ends, before the steady pattern settles.
                # Not worth running for all t: Tile occasionally schedules
                # a fill right before the tanh gate it feeds, stretching
                # that period by ~1.3us.
                if last_h is not None and 1 <= t <= 16:
                    fa, fb = rnn_ps[tn]
                    for i in range(12 if t <= 10 else 8):
                        fi = mm((fa if i % 2 == 0 else fb)[:, 0:BL],
                                dummy[:, :], dummy[:, 0:BL],
                                start=False, stop=False,
                                skip_group_check=True)
                        add_dep_helper(fi.ins, last_h.ins, sync=False,
                                       reason="early HAM fill")
                if t in delayed_loads:
                    for go in delayed_loads[t]:
                        go(last_h)
                hcur = hnew
                if debug and t in (0, 1):
                    nc.gpsimd.dma_start(out=dbg[f"dbg_h{t}"][:, :],
                                        in_=hcur[:, :])
            if debug:
                nc.gpsimd.dma_start(out=dbg["dbg_hlast"][:, :], in_=hcur[:, :])
            rnnps_ctx.__exit__(None, None, None)
            # tail pool in the 12KB the RNN PSUM pool just freed: enough
            # buffers that no layer's first matmul waits on a previous
            # layer's tanh to release its tile
            tail_ctx = tc.tile_pool(name="tailps", bufs=6, space="PSUM")
            tailps = tail_ctx.__enter__()

            def mlptile():
                return tailps.tile([128, 4 * BL], f32, name="tmlp")

            fps = tailps.tile([128, 64], f32, name="tmlp")  # shares slot ring

            def pefill(n):
                for _ in range(n):
                    mm(fps[:, :], dummy[:, :], dummy[:, 0:64],
                       start=True, stop=True, skip_group_check=True)

            # ---------------- h2o: r = tanh(h2o_W @ [u_last; h_last] + b) ----
            uc_last = ((T - 1) % CHUNK) * BL
            u_last_tile = u_tiles[(T - 1) // CHUNK]
            ps = mlptile()
            for m in range(2):
                mm(ps[:, BL * m:BL * (m + 1)],
                   h2ou_sb[:, 128 * m:128 * (m + 1)],
                   u_last_tile[:, uc_last:uc_last + BL], start=(m == 0),
                   stop=False, skip_group_check=True)
                for k in range(4):
                    j = k * 2 + m
                    mm(ps[:, BL * m:BL * (m + 1)],
                       h2o_sb[:, 128 * j:128 * (j + 1)],
                       hcur[:, BL * k:BL * (k + 1)],
                       start=False, stop=(k == 3), skip_group_check=True)
            r_sb = work.tile([128, 2 * BL], bf, name="r_sb")

            # ---------------- u_dnn tail --------------------------------
            # One wide ACT per PSUM tile (tail ACTs self-serialize at
            # ~400ns each, so fewer/wider beats 4x128-col), bias matmuls
            # emitted first (dep-free, they fill the tanh-latency windows),
            # and k-groups emitted per ACT half so layer N+1's matmuls
            # stream while layer N's second tanh is still running.
            nc.scalar.activation(r_sb[:, 0:2 * BL], ps[:, 0:2 * BL], TANH)
            pefill(12)

            ups1 = [mlptile(), mlptile()]
            for half in range(2):
                for mi in range(4):
                    m = half * 4 + mi
                    mm(ups1[half][:, BL * mi:BL * (mi + 1)],
                       u1tb_sb[:, 128 * m:128 * (m + 1)], tb_sb[:, :],
                       start=(mi == 0), stop=False, skip_group_check=True)
            for k in range(2):
                for half in range(2):
                    for mi in range(4):
                        m = half * 4 + mi
                        j = k * 8 + m
                        mm(ups1[half][:, BL * mi:BL * (mi + 1)],
                           u1_sb[:, 128 * j:128 * (j + 1)],
                           r_sb[:, BL * k:BL * (k + 1)],
                           start=False, stop=(k == 1), skip_group_check=True)
            ua1 = work.tile([128, 8 * BL], bf, name="ua1")
            nc.scalar.activation(ua1[:, 0:4 * BL], ups1[0][:, :], TANH)
            nc.scalar.activation(ua1[:, 4 * BL:8 * BL], ups1[1][:, :], TANH)
            pefill(8)

            ups2 = [mlptile(), mlptile()]
            for half in range(2):
                for mi in range(4):
                    m = half * 4 + mi
                    mm(ups2[half][:, BL * mi:BL * (mi + 1)],
                       u2b_sb[:, 128 * m:128 * (m + 1)], ones_sb[:, :],
                       start=(mi == 0), stop=False, skip_group_check=True)
            for k in range(8):
                for half in range(2):
                    for mi in range(4):
                        m = half * 4 + mi
                        j = k * 8 + m
                        mm(ups2[half][:, BL * mi:BL * (mi + 1)],
                           u2_sb[:, 128 * j:128 * (j + 1)],
                           ua1[:, BL * k:BL * (k + 1)],
                           start=False, stop=(k == 7), skip_group_check=True)
            ua2 = work.tile([128, 8 * BL], bf, name="ua2")
            nc.scalar.activation(ua2[:, 0:4 * BL], ups2[0][:, :], TANH)
            nc.scalar.activation(ua2[:, 4 * BL:8 * BL], ups2[1][:, :], TANH)
            pefill(10)

            ups3 = mlptile()
            for m in range(2):
                mm(ups3[:, BL * m:BL * (m + 1)],
                   u3b_sb[:, 128 * m:128 * (m + 1)], ones_sb[:, :],
                   start=(m == 0), stop=False, skip_group_check=True)
            for k in range(8):
                for m in range(2):
                    j = k * 2 + m
                    mm(ups3[:, BL * m:BL * (m + 1)],
                       u3_sb[:, 128 * j:128 * (j + 1)],
                       ua2[:, BL * k:BL * (k + 1)],
                       start=False, stop=(k == 7), skip_group_check=True)
            s_sb = xst["s_sb"]
            c_sb = work.tile([128, 2 * BL], bf, name="c_sb")
            nc.vector.tensor_copy(c_sb[:, :], ups3[:, 0:2 * BL])

            # ---------------- combinator ----------------
            # bias + state-part matmuls are c_sb-independent: they accumulate
            # while the ups3 -> c_sb copy is still in flight, leaving only
            # the 4 control-part matmuls on the final serial chain
            ps = mlptile()
            for m in range(2):
                o = ps[:, BL * m:BL * (m + 1)]
                mm(o, combb_sb[:, 128 * m:128 * (m + 1)], ones_sb[:, :],
                   start=(m == 0), stop=False, skip_group_check=True)
                for k in range(2):
                    j = k * 2 + m
                    mm(o, comb_sb[:, 128 * j:128 * (j + 1)],
                       s_sb[:, BL * k:BL * (k + 1)],
                       start=False, stop=False, skip_group_check=True)
            pefill(10)
            for m in range(2):
                o = ps[:, BL * m:BL * (m + 1)]
                for k in range(2, 4):
                    j = k * 2 + m
                    mm(o, comb_sb[:, 128 * j:128 * (j + 1)],
                       c_sb[:, BL * (k - 2):BL * (k - 1)],
                       start=False, stop=(k == 3), skip_group_check=True)
            out_sb = work.tile([128, 2 * BL], f32, name="out_sb")
            nc.vector.tensor_copy(out_sb[:, :], ps[:, 0:2 * BL])
            # two queues so the two halves of the output DMA overlap
            nc.sync.dma_start(out=out_d[0:128, :], in_=out_sb[:, 0:BL])
            nc.gpsimd.dma_start(out=out_d[128:256, :], in_=out_sb[:, BL:2 * BL])
            tail_ctx.__exit__(None, None, None)
            mlpps_ctx.__exit__(None, None, None)

    nc.compile()
    return nc


def _get_program():
    if "nc" not in _CACHE:
        _CACHE["nc"] = _build_program()
    return _CACHE["nc"]


def run(inputs, trace=False, trace_cores=None):
    from concourse.bass_utils import run_bass_kernel_spmd

    nc = _get_program()
    w = _weight_arrays(inputs)
    in_maps = []
    for c in range(NCORES):
        m = dict(w)
        m.update(_per_core_arrays(inputs, c))
        in_maps.append(m)
    res = run_bass_kernel_spmd(nc, in_maps, list(range(NCORES)),
                               trace=trace, trace_cores=trace_cores)
    out = np.empty((B, SD), np.float32)
    for c in range(NCORES):
        out[c * BL:(c + 1) * BL, :] = np.asarray(res.results[c]["out"]).T
    return out, res


def kernel(**inputs):
    out, _ = run(inputs)
    return out

